# revision 26
# baseline (speedup 1.0000x reference)
# Multi-head attention kernel for Trainium2 (Bass/Tile), SPMD over 8 cores.
#
# Problem (hardcoded shapes):
#   Wq [128, 8, 16], Wk [128, 8, 16], Wv [128, 16, 8], Wo [16, 8, 128],
#   vec [4, 2048, 128]  ->  out [4, 2048, 128]   (all float32)
#
# Sharding: core c handles batch c//2 and head-group c%2 (4 heads each).
# Each core computes its 4 heads' contribution to the output projection;
# the host sums the two head-group partials per batch.
#
# Per-core layout choices:
#  - scores are computed transposed, St[j, i] (j on partitions), so that the
#    softmax denominator sum_j exp(s) falls out of the AV matmul via an extra
#    ones-column in V ("V-hat"), and no PSUM transposes are needed.
#  - head h of the group lives at partition offset 32*h (PE row tiling), so
#    2 score matmuls run concurrently in the 128x128 PE array despite
#    head_dim=16. AV matmuls accumulate into 4 per-head PSUM banks (f32r
#    requires dst partition 0 and even stationary width, hence VW=32).
#  - exp() on the scalar engine is the bottleneck (16.8M elems/core); it reads
#    score PSUM tiles [128, 1024] directly and writes SBUF, one pass.

import numpy as np

B, N, UNIF, H, D = 4, 2048, 128, 8, 16
HG = 4         # heads per core
TI = 512       # i-tile width (query dim per inner tile)
TJ = 128       # j-tile width (key dim per matmul)
IT = N // TI   # 4 i-tiles
JT = N // TJ   # 16 j-tiles
VW = 32       # V-hat block width per head: col0=ones, 1..16=V, rest zero pad

_CACHE = {}


def _build_program():
    from contextlib import ExitStack

    import concourse.mybir as mybir
    import concourse.tile as tile
    from concourse import bacc

    f32 = mybir.dt.float32
    f32r = mybir.dt.float32r
    AF = mybir.ActivationFunctionType

    nc = bacc.Bacc("TRN2", target_bir_lowering=False, debug=False)

    vecb = nc.dram_tensor("vecb", [N, UNIF], f32, kind="ExternalInput").ap()
    wq = nc.dram_tensor("wq", [UNIF, HG * D], f32, kind="ExternalInput").ap()
    wk = nc.dram_tensor("wk", [UNIF, HG * D], f32, kind="ExternalInput").ap()
    wv = nc.dram_tensor("wv", [UNIF, HG * D], f32, kind="ExternalInput").ap()
    vinit = nc.dram_tensor("vinit", [128, JT * HG * VW], f32r, kind="ExternalInput").ap()
    wo = nc.dram_tensor("wo", [128, UNIF], f32r, kind="ExternalInput").ap()
    sel = nc.dram_tensor("sel", [128, 128], f32r, kind="ExternalInput").ap()
    ident = nc.dram_tensor("ident", [128, 128], f32, kind="ExternalInput").ap()
    out = nc.dram_tensor("out", [N, UNIF], f32, kind="ExternalOutput").ap()

    with tile.TileContext(nc) as tc, ExitStack() as ctx:
        consts = ctx.enter_context(tc.tile_pool(name="consts", bufs=1))
        big = ctx.enter_context(tc.tile_pool(name="big", bufs=1))
        vin = ctx.enter_context(tc.tile_pool(name="vin", bufs=4))
        epool = ctx.enter_context(tc.tile_pool(name="epool", bufs=3))
        post = ctx.enter_context(tc.tile_pool(name="post", bufs=2))
        ps = ctx.enter_context(tc.tile_pool(name="ps", bufs=2, space="PSUM"))
        avp = ctx.enter_context(tc.tile_pool(name="avp", bufs=4, space="PSUM"))

        # ---- constants ----
        wq_s = consts.tile([128, HG * D], f32)
        nc.sync.dma_start(out=wq_s, in_=wq)
        wk_s = consts.tile([128, HG * D], f32)
        nc.sync.dma_start(out=wk_s, in_=wk)
        wv_s = consts.tile([128, HG * D], f32)
        nc.sync.dma_start(out=wv_s, in_=wv)
        wo_s = consts.tile([128, UNIF], f32r)
        nc.sync.dma_start(out=wo_s, in_=wo)
        sel_s = consts.tile([128, 128], f32r)
        nc.sync.dma_start(out=sel_s, in_=sel)
        id_s = consts.tile([128, 128], f32)
        nc.sync.dma_start(out=id_s, in_=ident)

        # ---- persistent SBUF tensors ----
        vecT = big.tile([128, N], f32)              # vec^T [k, n]
        qt = big.tile([128, N], f32r)               # Q^T; head g at partitions 32g..32g+15
        kt = big.tile([128, N], f32r)               # K^T; same partition layout
        vhat = big.tile([128, JT * HG * VW], f32r)  # [j%128][jt][g][32]; col 0 = ones
        vhat4 = vhat.rearrange("p (jt g e) -> p jt g e", jt=JT, g=HG)
        nc.sync.dma_start(out=vhat, in_=vinit)

        # ---- load vec and transpose via PE (16x 128x128) ----
        for c4 in range(4):
            tp = ps.tile([128, 512], f32, tag="ps")
            for q in range(4):
                t = 4 * c4 + q
                vt = vin.tile([128, TJ], f32, tag="vin")
                nc.sync.dma_start(out=vt, in_=vecb[t * 128:(t + 1) * 128, :])
                nc.tensor.transpose(tp[:, q * 128:(q + 1) * 128], vt, id_s)
            nc.vector.tensor_copy(out=vecT[:, c4 * 512:(c4 + 1) * 512], in_=tp)

        # ---- Q/K projections (col-tiled: 4 heads concurrent per wave) ----
        for it4 in range(IT):
            qp = ps.tile([128, TI], f32, tag="ps")
            nc.vector.memset(qp, 0.0)
            for g in range(HG):
                nc.tensor.matmul(
                    qp[32 * g:32 * g + D, :],
                    lhsT=wq_s[:, g * D:(g + 1) * D],
                    rhs=vecT[:, it4 * TI:(it4 + 1) * TI],
                    start=True, stop=True,
                    tile_position=(0, 32 * g),
                )
            nc.vector.tensor_copy(out=qt[:, it4 * TI:(it4 + 1) * TI], in_=qp)
        for jt4 in range(IT):
            kp = ps.tile([128, TI], f32, tag="ps")
            nc.vector.memset(kp, 0.0)
            for g in range(HG):
                nc.tensor.matmul(
                    kp[32 * g:32 * g + D, :],
                    lhsT=wk_s[:, g * D:(g + 1) * D],
                    rhs=vecT[:, jt4 * TI:(jt4 + 1) * TI],
                    start=True, stop=True,
                    tile_position=(0, 32 * g),
                )
            nc.vector.tensor_copy(out=kt[:, jt4 * TI:(jt4 + 1) * TI], in_=kp)

        # ---- V projection: V[j, (g,d)] per j-tile, scattered into vhat ----
        for jt in range(JT):
            vp = ps.tile([128, HG * D], f32, tag="ps")
            nc.tensor.matmul(
                vp,
                lhsT=vecT[:, jt * TJ:(jt + 1) * TJ],
                rhs=wv_s,
                start=True, stop=True,
            )
            nc.vector.tensor_copy(
                out=vhat4[:, jt, :, 1:D + 1],
                in_=vp.rearrange("p (g d) -> p g d", g=HG),
            )

        # ---- main attention loop ----
        for it4 in range(IT):
            avs = []
            for g in range(HG):
                avg = avp.tile([32, TI], f32, tag="av", name=f"av{g}")
                avs.append(avg)
            for jt in range(JT):
                for w in range(2):
                    sc = ps.tile([128, 2 * TI], f32, tag="ps")
                    for hh in range(2):
                        g = 2 * w + hh
                        nc.tensor.matmul(
                            sc[:, hh * TI:(hh + 1) * TI],
                            lhsT=kt[32 * g:32 * g + D, jt * TJ:(jt + 1) * TJ],
                            rhs=qt[32 * g:32 * g + D, it4 * TI:(it4 + 1) * TI],
                            start=True, stop=True,
                            tile_position=(32 * g, 0),
                        )
                    ex = epool.tile([128, 2 * TI], f32r, tag="e")
                    nc.scalar.activation(out=ex, in_=sc, func=AF.Exp, scale=0.25)
                    for hh in range(2):
                        g = 2 * w + hh
                        nc.tensor.matmul(
                            avs[g],
                            lhsT=vhat4[:, jt, g, :],
                            rhs=ex[:, hh * TI:(hh + 1) * TI],
                            start=(jt == 0), stop=(jt == JT - 1),
                        )

            # ---- postlude: normalize + output projection ----
            ot = post.tile([128, TI], f32r, tag="ot")
            for g in range(HG):
                nc.vector.tensor_copy(out=ot[32 * g:32 * (g + 1), :], in_=avs[g])
            # broadcast each group's denominator row (partition 32g) to all of
            # the group's 32 partitions via a selection matmul, then invert.
            bb = ps.tile([128, TI], f32, tag="ps")
            nc.tensor.matmul(bb, lhsT=sel_s, rhs=ot, start=True, stop=True)
            rec = post.tile([128, TI], f32, tag="rec")
            nc.vector.reciprocal(out=rec, in_=bb)
            otn = post.tile([128, TI], f32r, tag="otn")
            nc.vector.tensor_mul(out=otn, in0=ot, in1=rec)
            for ic in range(4):
                fo = ps.tile([128, 128], f32, tag="ps")
                nc.tensor.matmul(
                    fo,
                    lhsT=otn[:, ic * 128:(ic + 1) * 128],
                    rhs=wo_s,
                    start=True, stop=True,
                )
                ob = post.tile([128, 128], f32, tag="ob")
                nc.vector.tensor_copy(out=ob, in_=fo)
                nc.sync.dma_start(
                    out=out[it4 * TI + ic * 128:it4 * TI + (ic + 1) * 128, :],
                    in_=ob,
                )

    nc.compile()
    return nc


def _prep_in_maps(Wq, Wk, Wv, Wo, vec):
    Wq = np.ascontiguousarray(Wq, np.float32)
    Wk = np.ascontiguousarray(Wk, np.float32)
    Wv = np.ascontiguousarray(Wv, np.float32)
    Wo = np.ascontiguousarray(Wo, np.float32)
    vec = np.ascontiguousarray(vec, np.float32)

    # sel.T @ x broadcasts partition row 32*(m//32) of x to every row m of
    # that 32-row group (used to spread softmax denominators to their heads).
    sel = np.zeros((128, 128), np.float32)
    for m in range(128):
        sel[32 * (m // 32), m] = 1.0
    # V-hat static pattern: ones column at offset 0 of each 32-wide block
    vinit = np.zeros((128, JT * HG * VW), np.float32)
    vinit[:, ::VW] = 1.0
    ident = np.eye(128, dtype=np.float32)

    grp_consts = []
    for grp in range(2):
        hs = slice(4 * grp, 4 * grp + 4)
        wq_g = Wq[:, hs, :].reshape(UNIF, HG * D)
        wk_g = Wk[:, hs, :].reshape(UNIF, HG * D)
        # wv free order (g, d):  wv_g[k, 16g+d] = Wv[k, d, 4*grp+g]
        wv_g = np.ascontiguousarray(
            Wv[:, :, hs].transpose(0, 2, 1)).reshape(UNIF, HG * D)
        # row 32g is the softmax-denominator row (killed by zeros); V values
        # sit at rows 32g+1 .. 32g+16 (ones-column-first V-hat layout).
        wo_g = np.zeros((128, UNIF), np.float32)
        for g in range(HG):
            wo_g[32 * g + 1:32 * g + 1 + D, :] = Wo[:, 4 * grp + g, :]
        grp_consts.append((wq_g, wk_g, wv_g, wo_g))

    in_maps = []
    for c in range(8):
        b, grp = c // 2, c % 2
        wq_g, wk_g, wv_g, wo_g = grp_consts[grp]
        in_maps.append({
            "vecb": np.ascontiguousarray(vec[b]),
            "wq": np.ascontiguousarray(wq_g),
            "wk": np.ascontiguousarray(wk_g),
            "wv": wv_g,
            "wo": wo_g,
            "sel": sel,
            "vinit": vinit,
            "ident": ident,
        })
    return in_maps


def _get_program():
    if "nc" not in _CACHE:
        _CACHE["nc"] = _build_program()
    return _CACHE["nc"]


def _run(inputs, trace=False, trace_kwargs=None):
    from concourse.bass_utils import run_bass_kernel_spmd

    nc = _get_program()
    in_maps = _prep_in_maps(**inputs)
    res = run_bass_kernel_spmd(
        nc, in_maps, core_ids=list(range(8)), trace=trace,
        **({"trace_kwargs": trace_kwargs} if trace_kwargs else {}),
    )
    _CACHE["last_results"] = res
    outs = [r["out"] for r in res.results]
    full = np.stack([outs[2 * b] + outs[2 * b + 1] for b in range(B)])
    return np.ascontiguousarray(full, np.float32)


def kernel(**inputs) -> np.ndarray:
    return _run(inputs, trace=False)


# revision 27
# speedup vs baseline: 1.3002x; 1.3002x over previous
# Multi-head attention kernel for Trainium2 (Bass/Tile), SPMD over 8 cores.
#
# Problem (hardcoded shapes):
#   Wq [128, 8, 16], Wk [128, 8, 16], Wv [128, 16, 8], Wo [16, 8, 128],
#   vec [4, 2048, 128]  ->  out [4, 2048, 128]   (all float32)
#
# Sharding: core c handles batch c//2 and head-group c%2 (4 heads each).
# Each core computes its 4 heads' contribution to the output projection;
# the host sums the two head-group partials per batch.
#
# Per-core layout choices:
#  - scores are computed transposed, St[j, i] (j on partitions), so that the
#    softmax denominator sum_j exp(s) falls out of the AV matmul via an extra
#    ones-column in V ("V-hat"), and no PSUM transposes are needed.
#  - head h of the group lives at partition offset 32*h (PE row tiling), so
#    2 score matmuls run concurrently in the 128x128 PE array despite
#    head_dim=16. AV matmuls accumulate into 4 per-head PSUM banks (f32r
#    requires dst partition 0 and even stationary width, hence VW=32).
#  - exp() on the scalar engine is the bottleneck (16.8M elems/core); it reads
#    score PSUM tiles [128, 1024] directly and writes SBUF, one pass.

import ml_dtypes
import numpy as np

B, N, UNIF, H, D = 4, 2048, 128, 8, 16
HG = 4         # heads per core
TI = 512       # i-tile width (query dim per inner tile)
TJ = 128       # j-tile width (key dim per matmul)
IT = N // TI   # 4 i-tiles
JT = N // TJ   # 16 j-tiles
VW = 32       # V-hat block width per head: col0=ones, 1..16=V, rest zero pad

_CACHE = {}


def _build_program():
    from contextlib import ExitStack

    import concourse.mybir as mybir
    import concourse.tile as tile
    from concourse import bacc

    f32 = mybir.dt.float32
    f32r = mybir.dt.float32r
    bf16 = mybir.dt.bfloat16
    AF = mybir.ActivationFunctionType

    nc = bacc.Bacc("TRN2", target_bir_lowering=False, debug=False)

    vecb = nc.dram_tensor("vecb", [N, UNIF], f32, kind="ExternalInput").ap()
    wq = nc.dram_tensor("wq", [UNIF, HG * D], f32, kind="ExternalInput").ap()
    wk = nc.dram_tensor("wk", [UNIF, HG * D], f32, kind="ExternalInput").ap()
    wv = nc.dram_tensor("wv", [UNIF, HG * D], f32, kind="ExternalInput").ap()
    vinit = nc.dram_tensor("vinit", [128, JT * HG * VW], bf16, kind="ExternalInput").ap()
    wo = nc.dram_tensor("wo", [128, UNIF], f32r, kind="ExternalInput").ap()
    sel = nc.dram_tensor("sel", [128, 128], f32r, kind="ExternalInput").ap()
    ident = nc.dram_tensor("ident", [128, 128], f32, kind="ExternalInput").ap()
    out = nc.dram_tensor("out", [N, UNIF], f32, kind="ExternalOutput").ap()

    with tile.TileContext(nc) as tc, ExitStack() as ctx:
        consts = ctx.enter_context(tc.tile_pool(name="consts", bufs=1))
        big = ctx.enter_context(tc.tile_pool(name="big", bufs=1))
        vin = ctx.enter_context(tc.tile_pool(name="vin", bufs=4))
        epool = ctx.enter_context(tc.tile_pool(name="epool", bufs=3))
        post = ctx.enter_context(tc.tile_pool(name="post", bufs=2))
        ps = ctx.enter_context(tc.tile_pool(name="ps", bufs=3, space="PSUM"))
        avp = ctx.enter_context(tc.tile_pool(name="avp", bufs=2, space="PSUM"))

        # ---- constants ----
        wq_s = consts.tile([128, HG * D], f32)
        nc.sync.dma_start(out=wq_s, in_=wq)
        wk_s = consts.tile([128, HG * D], f32)
        nc.sync.dma_start(out=wk_s, in_=wk)
        wv_s = consts.tile([128, HG * D], f32)
        nc.sync.dma_start(out=wv_s, in_=wv)
        wo_s = consts.tile([128, UNIF], f32r)
        nc.sync.dma_start(out=wo_s, in_=wo)
        sel_s = consts.tile([128, 128], f32r)
        nc.sync.dma_start(out=sel_s, in_=sel)
        id_s = consts.tile([128, 128], f32)
        nc.sync.dma_start(out=id_s, in_=ident)

        # ---- persistent SBUF tensors ----
        vecT = big.tile([128, N], f32)              # vec^T [k, n]
        qt = big.tile([128, N], f32r)               # Q^T; head g at partitions 32g..32g+15
        kt = big.tile([128, N], f32r)               # K^T; same partition layout
        vhat = big.tile([128, JT * HG * VW], bf16)  # [j%128][jt][g][32]; col 0 = ones
        vhat4 = vhat.rearrange("p (jt g e) -> p jt g e", jt=JT, g=HG)
        nc.sync.dma_start(out=vhat, in_=vinit)

        # ---- load vec and transpose via PE (16x 128x128) ----
        for c4 in range(4):
            tp = ps.tile([128, 512], f32, tag="ps")
            for q in range(4):
                t = 4 * c4 + q
                vt = vin.tile([128, TJ], f32, tag="vin")
                nc.sync.dma_start(out=vt, in_=vecb[t * 128:(t + 1) * 128, :])
                nc.tensor.transpose(tp[:, q * 128:(q + 1) * 128], vt, id_s)
            nc.vector.tensor_copy(out=vecT[:, c4 * 512:(c4 + 1) * 512], in_=tp)

        # ---- Q/K projections (col-tiled: 4 heads concurrent per wave) ----
        for it4 in range(IT):
            qp = ps.tile([128, TI], f32, tag="ps")
            nc.vector.memset(qp, 0.0)
            for g in range(HG):
                nc.tensor.matmul(
                    qp[32 * g:32 * g + D, :],
                    lhsT=wq_s[:, g * D:(g + 1) * D],
                    rhs=vecT[:, it4 * TI:(it4 + 1) * TI],
                    start=True, stop=True,
                    tile_position=(0, 32 * g),
                )
            nc.vector.tensor_copy(out=qt[:, it4 * TI:(it4 + 1) * TI], in_=qp)
        for jt4 in range(IT):
            kp = ps.tile([128, TI], f32, tag="ps")
            nc.vector.memset(kp, 0.0)
            for g in range(HG):
                nc.tensor.matmul(
                    kp[32 * g:32 * g + D, :],
                    lhsT=wk_s[:, g * D:(g + 1) * D],
                    rhs=vecT[:, jt4 * TI:(jt4 + 1) * TI],
                    start=True, stop=True,
                    tile_position=(0, 32 * g),
                )
            nc.vector.tensor_copy(out=kt[:, jt4 * TI:(jt4 + 1) * TI], in_=kp)

        # ---- V projection: V[j, (g,d)] per j-tile, scattered into vhat ----
        for jt in range(JT):
            vp = ps.tile([128, HG * D], f32, tag="ps")
            nc.tensor.matmul(
                vp,
                lhsT=vecT[:, jt * TJ:(jt + 1) * TJ],
                rhs=wv_s,
                start=True, stop=True,
            )
            nc.vector.tensor_copy(
                out=vhat4[:, jt, :, 1:D + 1],
                in_=vp.rearrange("p (g d) -> p g d", g=HG),
            )

        # ---- main attention loop ----
        for it4 in range(IT):
            avt = avp.tile([128, TI], f32, tag="av")
            nc.vector.memset(avt, 0.0)
            for jt in range(JT):
                # both score waves first: PE always has independent work
                # queued while ACT drains the previous wave's exp
                scs = []
                for w in range(2):
                    sc = ps.tile([128, 2 * TI], f32, tag="ps", name=f"sc{w}")
                    for hh in range(2):
                        g = 2 * w + hh
                        nc.tensor.matmul(
                            sc[:, hh * TI:(hh + 1) * TI],
                            lhsT=kt[32 * g:32 * g + D, jt * TJ:(jt + 1) * TJ],
                            rhs=qt[32 * g:32 * g + D, it4 * TI:(it4 + 1) * TI],
                            start=True, stop=True,
                            tile_position=(32 * g, 0),
                        )
                    scs.append(sc)
                exs = []
                for w in range(2):
                    ex = epool.tile([128, 2 * TI], bf16, tag="e", name=f"ex{w}")
                    nc.scalar.activation(out=ex, in_=scs[w], func=AF.Exp, scale=0.25)
                    exs.append(ex)
                for w in range(2):
                    for hh in range(2):
                        g = 2 * w + hh
                        nc.tensor.matmul(
                            avt[32 * g:32 * g + VW, :],
                            lhsT=vhat4[:, jt, g, :],
                            rhs=exs[w][:, hh * TI:(hh + 1) * TI],
                            start=(jt == 0 and g == 0), stop=(jt == JT - 1),
                            tile_position=(0, 32 * g),
                            skip_group_check=(g > 0),
                        )

            # ---- postlude: normalize + output projection ----
            ot = post.tile([128, TI], f32r, tag="ot")
            nc.vector.tensor_copy(out=ot, in_=avt)
            # broadcast each group's denominator row (partition 32g) to all of
            # the group's 32 partitions via a selection matmul, then invert.
            bb = ps.tile([128, TI], f32, tag="ps")
            nc.tensor.matmul(bb, lhsT=sel_s, rhs=ot, start=True, stop=True)
            rec = post.tile([128, TI], f32, tag="rec")
            nc.vector.reciprocal(out=rec, in_=bb)
            otn = post.tile([128, TI], f32r, tag="otn")
            nc.vector.tensor_mul(out=otn, in0=ot, in1=rec)
            for ic in range(4):
                fo = ps.tile([128, 128], f32, tag="ps")
                nc.tensor.matmul(
                    fo,
                    lhsT=otn[:, ic * 128:(ic + 1) * 128],
                    rhs=wo_s,
                    start=True, stop=True,
                )
                ob = post.tile([128, 128], f32, tag="ob")
                nc.vector.tensor_copy(out=ob, in_=fo)
                nc.sync.dma_start(
                    out=out[it4 * TI + ic * 128:it4 * TI + (ic + 1) * 128, :],
                    in_=ob,
                )

    nc.compile()
    return nc


def _prep_in_maps(Wq, Wk, Wv, Wo, vec):
    Wq = np.ascontiguousarray(Wq, np.float32)
    Wk = np.ascontiguousarray(Wk, np.float32)
    Wv = np.ascontiguousarray(Wv, np.float32)
    Wo = np.ascontiguousarray(Wo, np.float32)
    vec = np.ascontiguousarray(vec, np.float32)

    # sel.T @ x broadcasts partition row 32*(m//32) of x to every row m of
    # that 32-row group (used to spread softmax denominators to their heads).
    sel = np.zeros((128, 128), np.float32)
    for m in range(128):
        sel[32 * (m // 32), m] = 1.0
    # V-hat static pattern: ones column at offset 0 of each 32-wide block
    vinit = np.zeros((128, JT * HG * VW), np.float32)
    vinit[:, ::VW] = 1.0
    vinit = vinit.astype(ml_dtypes.bfloat16)
    ident = np.eye(128, dtype=np.float32)

    grp_consts = []
    for grp in range(2):
        hs = slice(4 * grp, 4 * grp + 4)
        wq_g = Wq[:, hs, :].reshape(UNIF, HG * D)
        wk_g = Wk[:, hs, :].reshape(UNIF, HG * D)
        # wv free order (g, d):  wv_g[k, 16g+d] = Wv[k, d, 4*grp+g]
        wv_g = np.ascontiguousarray(
            Wv[:, :, hs].transpose(0, 2, 1)).reshape(UNIF, HG * D)
        # row 32g is the softmax-denominator row (killed by zeros); V values
        # sit at rows 32g+1 .. 32g+16 (ones-column-first V-hat layout).
        wo_g = np.zeros((128, UNIF), np.float32)
        for g in range(HG):
            wo_g[32 * g + 1:32 * g + 1 + D, :] = Wo[:, 4 * grp + g, :]
        grp_consts.append((wq_g, wk_g, wv_g, wo_g))

    in_maps = []
    for c in range(8):
        b, grp = c // 2, c % 2
        wq_g, wk_g, wv_g, wo_g = grp_consts[grp]
        in_maps.append({
            "vecb": np.ascontiguousarray(vec[b]),
            "wq": np.ascontiguousarray(wq_g),
            "wk": np.ascontiguousarray(wk_g),
            "wv": wv_g,
            "wo": wo_g,
            "sel": sel,
            "vinit": vinit,
            "ident": ident,
        })
    return in_maps


def _get_program():
    if "nc" not in _CACHE:
        _CACHE["nc"] = _build_program()
    return _CACHE["nc"]


def _run(inputs, trace=False, trace_kwargs=None):
    from concourse.bass_utils import run_bass_kernel_spmd

    nc = _get_program()
    in_maps = _prep_in_maps(**inputs)
    res = run_bass_kernel_spmd(
        nc, in_maps, core_ids=list(range(8)), trace=trace,
        **({"trace_kwargs": trace_kwargs} if trace_kwargs else {}),
    )
    _CACHE["last_results"] = res
    outs = [r["out"] for r in res.results]
    full = np.stack([outs[2 * b] + outs[2 * b + 1] for b in range(B)])
    return np.ascontiguousarray(full, np.float32)


def kernel(**inputs) -> np.ndarray:
    return _run(inputs, trace=False)


# revision 29
# speedup vs baseline: 1.4487x; 1.1142x over previous
# Multi-head attention kernel for Trainium2 (Bass/Tile), SPMD over 8 cores.
#
# Problem (hardcoded shapes):
#   Wq [128, 8, 16], Wk [128, 8, 16], Wv [128, 16, 8], Wo [16, 8, 128],
#   vec [4, 2048, 128]  ->  out [4, 2048, 128]   (all float32)
#
# Sharding: core c handles batch c//2 and head-group c%2 (4 heads each).
# Each core computes its 4 heads' contribution to the output projection;
# the host sums the two head-group partials per batch.
#
# Per-core layout choices:
#  - scores are computed transposed, St[j, i] (j on partitions), so that the
#    softmax denominator sum_j exp(s) falls out of the AV matmul via an extra
#    ones-column in V ("V-hat"), and no PSUM transposes are needed.
#  - head h of the group lives at partition offset 32*h (PE row tiling), so
#    2 score matmuls run concurrently in the 128x128 PE array despite
#    head_dim=16. AV matmuls accumulate into 4 per-head PSUM banks (f32r
#    requires dst partition 0 and even stationary width, hence VW=32).
#  - exp() on the scalar engine is the bottleneck (16.8M elems/core); it reads
#    score PSUM tiles [128, 1024] directly and writes SBUF, one pass.

import ml_dtypes
import numpy as np

B, N, UNIF, H, D = 4, 2048, 128, 8, 16
HG = 4         # heads per core
TI = 512       # i-tile width (query dim per inner tile)
TJ = 128       # j-tile width (key dim per matmul)
IT = N // TI   # 4 i-tiles
JT = N // TJ   # 16 j-tiles
VW = 32       # V-hat block width per head: col0=ones, 1..16=V, rest zero pad

_CACHE = {}


def _build_program():
    from contextlib import ExitStack

    import concourse.mybir as mybir
    import concourse.tile as tile
    from concourse import bacc

    f32 = mybir.dt.float32
    f32r = mybir.dt.float32r
    bf16 = mybir.dt.bfloat16
    AF = mybir.ActivationFunctionType

    nc = bacc.Bacc("TRN2", target_bir_lowering=False, debug=False)

    vecb = nc.dram_tensor("vecb", [N, UNIF], f32, kind="ExternalInput").ap()
    wq = nc.dram_tensor("wq", [UNIF, HG * D], f32, kind="ExternalInput").ap()
    wk = nc.dram_tensor("wk", [UNIF, HG * D], f32, kind="ExternalInput").ap()
    wv = nc.dram_tensor("wv", [UNIF, HG * D], f32, kind="ExternalInput").ap()
    vinit = nc.dram_tensor("vinit", [128, JT * HG * VW], bf16, kind="ExternalInput").ap()
    wo = nc.dram_tensor("wo", [128, UNIF], f32r, kind="ExternalInput").ap()
    sel = nc.dram_tensor("sel", [128, 128], f32r, kind="ExternalInput").ap()
    ident = nc.dram_tensor("ident", [128, 128], f32, kind="ExternalInput").ap()
    out = nc.dram_tensor("out", [N, UNIF], f32, kind="ExternalOutput").ap()

    with tile.TileContext(nc) as tc, ExitStack() as ctx:
        consts = ctx.enter_context(tc.tile_pool(name="consts", bufs=1))
        big = ctx.enter_context(tc.tile_pool(name="big", bufs=1))
        vin = ctx.enter_context(tc.tile_pool(name="vin", bufs=4))
        epool = ctx.enter_context(tc.tile_pool(name="epool", bufs=5))
        post = ctx.enter_context(tc.tile_pool(name="post", bufs=2))
        ps = ctx.enter_context(tc.tile_pool(name="ps", bufs=3, space="PSUM"))
        avp = ctx.enter_context(tc.tile_pool(name="avp", bufs=2, space="PSUM"))

        # ---- constants ----
        wq_s = consts.tile([128, HG * D], f32)
        nc.sync.dma_start(out=wq_s, in_=wq)
        wk_s = consts.tile([128, HG * D], f32)
        nc.sync.dma_start(out=wk_s, in_=wk)
        wv_s = consts.tile([128, HG * D], f32)
        nc.sync.dma_start(out=wv_s, in_=wv)
        wo_s = consts.tile([128, UNIF], f32r)
        nc.sync.dma_start(out=wo_s, in_=wo)
        sel_s = consts.tile([128, 128], f32r)
        nc.sync.dma_start(out=sel_s, in_=sel)
        id_s = consts.tile([128, 128], f32)
        nc.sync.dma_start(out=id_s, in_=ident)

        # ---- persistent SBUF tensors ----
        vecT = big.tile([128, N], f32)              # vec^T [k, n]
        qt = big.tile([128, N], f32r)               # Q^T; head g at partitions 32g..32g+15
        kt = big.tile([128, N], f32r)               # K^T; same partition layout
        vhat = big.tile([128, JT * HG * VW], bf16)  # [j%128][jt][g][32]; col 0 = ones
        vhat4 = vhat.rearrange("p (jt g e) -> p jt g e", jt=JT, g=HG)
        nc.sync.dma_start(out=vhat, in_=vinit)

        # ---- load vec and transpose via PE (16x 128x128) ----
        for c4 in range(4):
            tp = ps.tile([128, 512], f32, tag="ps")
            for q in range(4):
                t = 4 * c4 + q
                vt = vin.tile([128, TJ], f32, tag="vin")
                nc.sync.dma_start(out=vt, in_=vecb[t * 128:(t + 1) * 128, :])
                nc.tensor.transpose(tp[:, q * 128:(q + 1) * 128], vt, id_s)
            nc.vector.tensor_copy(out=vecT[:, c4 * 512:(c4 + 1) * 512], in_=tp)

        # ---- Q/K projections (col-tiled: 4 heads concurrent per wave) ----
        for it4 in range(IT):
            qp = ps.tile([128, TI], f32, tag="ps")
            nc.vector.memset(qp, 0.0)
            for g in range(HG):
                nc.tensor.matmul(
                    qp[32 * g:32 * g + D, :],
                    lhsT=wq_s[:, g * D:(g + 1) * D],
                    rhs=vecT[:, it4 * TI:(it4 + 1) * TI],
                    start=True, stop=True,
                    tile_position=(0, 32 * g),
                )
            nc.vector.tensor_copy(out=qt[:, it4 * TI:(it4 + 1) * TI], in_=qp)
        for jt4 in range(IT):
            kp = ps.tile([128, TI], f32, tag="ps")
            nc.vector.memset(kp, 0.0)
            for g in range(HG):
                nc.tensor.matmul(
                    kp[32 * g:32 * g + D, :],
                    lhsT=wk_s[:, g * D:(g + 1) * D],
                    rhs=vecT[:, jt4 * TI:(jt4 + 1) * TI],
                    start=True, stop=True,
                    tile_position=(0, 32 * g),
                )
            nc.vector.tensor_copy(out=kt[:, jt4 * TI:(jt4 + 1) * TI], in_=kp)

        # ---- V projection: V[j, (g,d)] per j-tile, scattered into vhat ----
        for jt in range(JT):
            vp = ps.tile([128, HG * D], f32, tag="ps")
            nc.tensor.matmul(
                vp,
                lhsT=vecT[:, jt * TJ:(jt + 1) * TJ],
                rhs=wv_s,
                start=True, stop=True,
            )
            nc.vector.tensor_copy(
                out=vhat4[:, jt, :, 1:D + 1],
                in_=vp.rearrange("p (g d) -> p g d", g=HG),
            )

        # ---- main attention loop ----
        for it4 in range(IT):
            avt = avp.tile([128, TI], f32, tag="av")
            nc.vector.memset(avt, 0.0)
            # software pipeline across j-tiles: emit scores(jt) then AV(jt-1)
            # so the PE queue never blocks on the exp of the current j-tile.
            pend = None  # exp tiles of the previous j-tile
            for jt in range(JT + 1):
                if jt < JT:
                    exs = []
                    for w in range(2):
                        sc = ps.tile([128, 2 * TI], f32, tag="ps", name=f"sc{w}")
                        for hh in range(2):
                            g = 2 * w + hh
                            nc.tensor.matmul(
                                sc[:, hh * TI:(hh + 1) * TI],
                                lhsT=kt[32 * g:32 * g + D, jt * TJ:(jt + 1) * TJ],
                                rhs=qt[32 * g:32 * g + D, it4 * TI:(it4 + 1) * TI],
                                start=True, stop=True,
                                tile_position=(32 * g, 0),
                            )
                        ex = epool.tile([128, 2 * TI], bf16, tag="e", name=f"ex{w}")
                        nc.scalar.activation(out=ex, in_=sc, func=AF.Exp, scale=0.25)
                        exs.append(ex)
                if pend is not None:
                    pjt = jt - 1
                    for w in range(2):
                        for hh in range(2):
                            g = 2 * w + hh
                            nc.tensor.matmul(
                                avt[32 * g:32 * g + VW, :],
                                lhsT=vhat4[:, pjt, g, :],
                                rhs=pend[w][:, hh * TI:(hh + 1) * TI],
                                start=(pjt == 0 and g == 0), stop=(pjt == JT - 1),
                                tile_position=(0, 32 * g),
                                skip_group_check=(g > 0),
                            )
                pend = exs if jt < JT else None

            # ---- postlude: normalize + output projection ----
            ot = post.tile([128, TI], f32r, tag="ot")
            nc.vector.tensor_copy(out=ot, in_=avt)
            # broadcast each group's denominator row (partition 32g) to all of
            # the group's 32 partitions via a selection matmul, then invert.
            bb = ps.tile([128, TI], f32, tag="ps")
            nc.tensor.matmul(bb, lhsT=sel_s, rhs=ot, start=True, stop=True)
            rec = post.tile([128, TI], f32, tag="rec")
            nc.vector.reciprocal(out=rec, in_=bb)
            otn = post.tile([128, TI], f32r, tag="otn")
            nc.vector.tensor_mul(out=otn, in0=ot, in1=rec)
            for ic in range(4):
                fo = ps.tile([128, 128], f32, tag="ps")
                nc.tensor.matmul(
                    fo,
                    lhsT=otn[:, ic * 128:(ic + 1) * 128],
                    rhs=wo_s,
                    start=True, stop=True,
                )
                ob = post.tile([128, 128], f32, tag="ob")
                nc.vector.tensor_copy(out=ob, in_=fo)
                nc.sync.dma_start(
                    out=out[it4 * TI + ic * 128:it4 * TI + (ic + 1) * 128, :],
                    in_=ob,
                )

    nc.compile()
    return nc


def _prep_in_maps(Wq, Wk, Wv, Wo, vec):
    Wq = np.ascontiguousarray(Wq, np.float32)
    Wk = np.ascontiguousarray(Wk, np.float32)
    Wv = np.ascontiguousarray(Wv, np.float32)
    Wo = np.ascontiguousarray(Wo, np.float32)
    vec = np.ascontiguousarray(vec, np.float32)

    # sel.T @ x broadcasts partition row 32*(m//32) of x to every row m of
    # that 32-row group (used to spread softmax denominators to their heads).
    sel = np.zeros((128, 128), np.float32)
    for m in range(128):
        sel[32 * (m // 32), m] = 1.0
    # V-hat static pattern: ones column at offset 0 of each 32-wide block
    vinit = np.zeros((128, JT * HG * VW), np.float32)
    vinit[:, ::VW] = 1.0
    vinit = vinit.astype(ml_dtypes.bfloat16)
    ident = np.eye(128, dtype=np.float32)

    grp_consts = []
    for grp in range(2):
        hs = slice(4 * grp, 4 * grp + 4)
        wq_g = Wq[:, hs, :].reshape(UNIF, HG * D)
        wk_g = Wk[:, hs, :].reshape(UNIF, HG * D)
        # wv free order (g, d):  wv_g[k, 16g+d] = Wv[k, d, 4*grp+g]
        wv_g = np.ascontiguousarray(
            Wv[:, :, hs].transpose(0, 2, 1)).reshape(UNIF, HG * D)
        # row 32g is the softmax-denominator row (killed by zeros); V values
        # sit at rows 32g+1 .. 32g+16 (ones-column-first V-hat layout).
        wo_g = np.zeros((128, UNIF), np.float32)
        for g in range(HG):
            wo_g[32 * g + 1:32 * g + 1 + D, :] = Wo[:, 4 * grp + g, :]
        grp_consts.append((wq_g, wk_g, wv_g, wo_g))

    in_maps = []
    for c in range(8):
        b, grp = c // 2, c % 2
        wq_g, wk_g, wv_g, wo_g = grp_consts[grp]
        in_maps.append({
            "vecb": np.ascontiguousarray(vec[b]),
            "wq": np.ascontiguousarray(wq_g),
            "wk": np.ascontiguousarray(wk_g),
            "wv": wv_g,
            "wo": wo_g,
            "sel": sel,
            "vinit": vinit,
            "ident": ident,
        })
    return in_maps


def _get_program():
    if "nc" not in _CACHE:
        _CACHE["nc"] = _build_program()
    return _CACHE["nc"]


def _run(inputs, trace=False, trace_kwargs=None):
    from concourse.bass_utils import run_bass_kernel_spmd

    nc = _get_program()
    in_maps = _prep_in_maps(**inputs)
    res = run_bass_kernel_spmd(
        nc, in_maps, core_ids=list(range(8)), trace=trace,
        **({"trace_kwargs": trace_kwargs} if trace_kwargs else {}),
    )
    _CACHE["last_results"] = res
    outs = [r["out"] for r in res.results]
    full = np.stack([outs[2 * b] + outs[2 * b + 1] for b in range(B)])
    return np.ascontiguousarray(full, np.float32)


def kernel(**inputs) -> np.ndarray:
    return _run(inputs, trace=False)


# revision 30
# speedup vs baseline: 1.5338x; 1.0587x over previous
# Multi-head attention kernel for Trainium2 (Bass/Tile), SPMD over 8 cores.
#
# Problem (hardcoded shapes):
#   Wq [128, 8, 16], Wk [128, 8, 16], Wv [128, 16, 8], Wo [16, 8, 128],
#   vec [4, 2048, 128]  ->  out [4, 2048, 128]   (all float32)
#
# Sharding: core c handles batch c//2 and head-group c%2 (4 heads each).
# Each core computes its 4 heads' contribution to the output projection;
# the host sums the two head-group partials per batch.
#
# Per-core layout choices:
#  - scores are computed transposed, St[j, i] (j on partitions), so that the
#    softmax denominator sum_j exp(s) falls out of the AV matmul via an extra
#    ones-column in V ("V-hat"), and no PSUM transposes are needed.
#  - head h of the group lives at partition offset 32*h (PE row tiling), so
#    2 score matmuls run concurrently in the 128x128 PE array despite
#    head_dim=16. AV matmuls accumulate into 4 per-head PSUM banks (f32r
#    requires dst partition 0 and even stationary width, hence VW=32).
#  - exp() on the scalar engine is the bottleneck (16.8M elems/core); it reads
#    score PSUM tiles [128, 1024] directly and writes SBUF, one pass.

import ml_dtypes
import numpy as np

B, N, UNIF, H, D = 4, 2048, 128, 8, 16
HG = 4         # heads per core
TI = 512       # i-tile width (query dim per inner tile)
TJ = 128       # j-tile width (key dim per matmul)
IT = N // TI   # 4 i-tiles
JT = N // TJ   # 16 j-tiles
VW = 32       # V-hat block width per head: col0=ones, 1..16=V, rest zero pad

_CACHE = {}


def _build_program():
    from contextlib import ExitStack

    import concourse.mybir as mybir
    import concourse.tile as tile
    from concourse import bacc

    f32 = mybir.dt.float32
    f32r = mybir.dt.float32r
    f16 = mybir.dt.float16
    AF = mybir.ActivationFunctionType

    nc = bacc.Bacc("TRN2", target_bir_lowering=False, debug=False)

    vecb = nc.dram_tensor("vecb", [N, UNIF], f32, kind="ExternalInput").ap()
    amat = nc.dram_tensor("amat", [128, HG * 128], f32r, kind="ExternalInput").ap()
    wv = nc.dram_tensor("wv", [UNIF, HG * D], f32r, kind="ExternalInput").ap()
    vinit = nc.dram_tensor("vinit", [128, JT * HG * VW], f16, kind="ExternalInput").ap()
    wo = nc.dram_tensor("wo", [128, UNIF], f32r, kind="ExternalInput").ap()
    sel = nc.dram_tensor("sel", [128, 128], f32r, kind="ExternalInput").ap()
    ident = nc.dram_tensor("ident", [128, 128], f32, kind="ExternalInput").ap()
    out = nc.dram_tensor("out", [N, UNIF], f32, kind="ExternalOutput").ap()

    with tile.TileContext(nc) as tc, ExitStack() as ctx:
        consts = ctx.enter_context(tc.tile_pool(name="consts", bufs=1))
        big = ctx.enter_context(tc.tile_pool(name="big", bufs=1))
        epool = ctx.enter_context(tc.tile_pool(name="epool", bufs=5))
        post = ctx.enter_context(tc.tile_pool(name="post", bufs=2))
        ps = ctx.enter_context(tc.tile_pool(name="ps", bufs=3, space="PSUM"))
        avp = ctx.enter_context(tc.tile_pool(name="avp", bufs=2, space="PSUM"))

        # ---- constants ----
        amat_s = consts.tile([128, HG * 128], f32r)
        nc.sync.dma_start(out=amat_s, in_=amat)
        wv_s = consts.tile([128, HG * D], f32r)
        nc.sync.dma_start(out=wv_s, in_=wv)
        wo_s = consts.tile([128, UNIF], f32r)
        nc.sync.dma_start(out=wo_s, in_=wo)
        sel_s = consts.tile([128, 128], f32r)
        nc.sync.dma_start(out=sel_s, in_=sel)
        id_s = consts.tile([128, 128], f32)
        nc.sync.dma_start(out=id_s, in_=ident)

        # ---- persistent SBUF tensors ----
        vec_in = big.tile([128, N], f32)            # vec rows tiled: [p][t*128+k]
        vecT = big.tile([128, N], f32r)             # vec^T [k, n]
        ct = big.tile([128, HG * N], f32r)          # Ct_g = (vec @ A_g)^T, [c, n]
        vhat = big.tile([128, JT * HG * VW], f16)   # [j%128][jt][g][32]; col 0 = ones
        vhat4 = vhat.rearrange("p (jt g e) -> p jt g e", jt=JT, g=HG)
        nc.sync.dma_start(out=vhat, in_=vinit)

        # ---- load vec (2 bulk DMAs) and transpose via PE (16x 128x128) ----
        vec3 = vec_in.rearrange("p (t k) -> p t k", k=TJ)
        vsrc = vecb.rearrange("(t p) k -> p t k", p=128)
        for half in range(2):
            nc.sync.dma_start(out=vec3[:, half * 8:(half + 1) * 8, :],
                              in_=vsrc[:, half * 8:(half + 1) * 8, :])
        for c4 in range(4):
            tp = ps.tile([128, 512], f32, tag="ps")
            for q in range(4):
                t = 4 * c4 + q
                nc.tensor.transpose(tp[:, q * 128:(q + 1) * 128], vec3[:, t, :], id_s)
            nc.vector.tensor_copy(out=vecT[:, c4 * 512:(c4 + 1) * 512], in_=tp)

        # ---- Ct_g = A_g^T-contracted projection: one [128,512] chunk at a time
        for g in range(HG):
            for c4 in range(IT):
                cp = ps.tile([128, TI], f32, tag="ps")
                nc.tensor.matmul(
                    cp,
                    lhsT=amat_s[:, g * 128:(g + 1) * 128],
                    rhs=vecT[:, c4 * TI:(c4 + 1) * TI],
                    start=True, stop=True,
                )
                nc.vector.tensor_copy(
                    out=ct[:, g * N + c4 * TI:g * N + (c4 + 1) * TI], in_=cp)

        # ---- V projection: V[j, (g,d)] per j-tile, scattered into vhat ----
        for jt in range(JT):
            vp = ps.tile([128, HG * D], f32, tag="ps")
            nc.tensor.matmul(
                vp,
                lhsT=vecT[:, jt * TJ:(jt + 1) * TJ],
                rhs=wv_s,
                start=True, stop=True,
            )
            nc.vector.tensor_copy(
                out=vhat4[:, jt, :, 1:D + 1],
                in_=vp.rearrange("p (g d) -> p g d", g=HG),
            )

        # ---- main attention loop (postlude deferred into the next i-tile
        #      so its PE ops never starve the scalar engine) ----
        pending_post = [None]

        def postlude(avt, it4):
            ot = post.tile([128, TI], f32r, tag="ot", name="ot")
            nc.vector.tensor_copy(out=ot, in_=avt)
            bb = ps.tile([128, TI], f32, tag="ps", name="bb")
            nc.tensor.matmul(bb, lhsT=sel_s, rhs=ot, start=True, stop=True)
            rec = post.tile([128, TI], f32, tag="rec", name="rec")
            nc.vector.reciprocal(out=rec, in_=bb)
            otn = post.tile([128, TI], f32r, tag="otn", name="otn")
            nc.vector.tensor_mul(out=otn, in0=ot, in1=rec)
            for ic in range(4):
                fo = ps.tile([128, 128], f32, tag="ps", name="fo")
                nc.tensor.matmul(
                    fo,
                    lhsT=otn[:, ic * 128:(ic + 1) * 128],
                    rhs=wo_s,
                    start=True, stop=True,
                )
                ob = post.tile([128, 128], f32, tag="ob", name="ob")
                nc.vector.tensor_copy(out=ob, in_=fo)
                nc.sync.dma_start(
                    out=out[it4 * TI + ic * 128:it4 * TI + (ic + 1) * 128, :],
                    in_=ob,
                )

        for it4 in range(IT):
            avt = avp.tile([128, TI], f32, tag="av")
            nc.vector.memset(avt, 0.0)
            # software pipeline across j-tiles: emit scores(jt) then AV(jt-1)
            pend = None
            for jt in range(JT + 1):
                if jt < JT:
                    exs = []
                    for w in range(2):
                        sc = ps.tile([128, 2 * TI], f32, tag="ps", name=f"sc{w}")
                        for hh in range(2):
                            g = 2 * w + hh
                            nc.tensor.matmul(
                                sc[:, hh * TI:(hh + 1) * TI],
                                lhsT=ct[:, g * N + jt * TJ:g * N + (jt + 1) * TJ],
                                rhs=vecT[:, it4 * TI:(it4 + 1) * TI],
                                start=True, stop=True,
                            )
                        ex = epool.tile([128, 2 * TI], f16, tag="e", name=f"ex{w}")
                        nc.scalar.activation(out=ex, in_=sc, func=AF.Exp, scale=0.25)
                        exs.append(ex)
                if pend is not None:
                    pjt = jt - 1
                    for w in range(2):
                        for hh in range(2):
                            g = 2 * w + hh
                            nc.tensor.matmul(
                                avt[32 * g:32 * g + VW, :],
                                lhsT=vhat4[:, pjt, g, :],
                                rhs=pend[w][:, hh * TI:(hh + 1) * TI],
                                start=(pjt == 0 and g == 0), stop=(pjt == JT - 1),
                                tile_position=(0, 32 * g),
                                skip_group_check=(g > 0),
                            )
                pend = exs if jt < JT else None
                # after this i-tile's first scores are queued, flush the
                # previous i-tile's postlude
                if jt == 2 and pending_post[0] is not None:
                    pending_post[0]()
                    pending_post[0] = None
            pending_post[0] = (lambda a=avt, i=it4: postlude(a, i))
        pending_post[0]()

    nc.compile()
    return nc


def _prep_in_maps(Wq, Wk, Wv, Wo, vec):
    Wq = np.ascontiguousarray(Wq, np.float32)
    Wk = np.ascontiguousarray(Wk, np.float32)
    Wv = np.ascontiguousarray(Wv, np.float32)
    Wo = np.ascontiguousarray(Wo, np.float32)
    vec = np.ascontiguousarray(vec, np.float32)

    # sel.T @ x broadcasts partition row 32*(m//32) of x to every row m of
    # that 32-row group (used to spread softmax denominators to their heads).
    sel = np.zeros((128, 128), np.float32)
    for m in range(128):
        sel[32 * (m // 32), m] = 1.0
    # V-hat static pattern: ones column at offset 0 of each 32-wide block
    vinit = np.zeros((128, JT * HG * VW), np.float32)
    vinit[:, ::VW] = 1.0
    vinit = vinit.astype(np.float16)
    ident = np.eye(128, dtype=np.float32)

    grp_consts = []
    for grp in range(2):
        hs = slice(4 * grp, 4 * grp + 4)
        # scores are computed as vec @ A_h @ vec^T with A_h = Wk_h Wq_h^T,
        # so S^T[j,i] = k_j . q_i  (precomputed on host in float64)
        amat = np.zeros((128, HG * 128), np.float32)
        for g in range(HG):
            h = 4 * grp + g
            A = Wk[:, h, :].astype(np.float64) @ Wq[:, h, :].astype(np.float64).T
            amat[:, g * 128:(g + 1) * 128] = A.astype(np.float32)
        # wv free order (g, d):  wv_g[k, 16g+d] = Wv[k, d, 4*grp+g]
        wv_g = np.ascontiguousarray(
            Wv[:, :, hs].transpose(0, 2, 1)).reshape(UNIF, HG * D)
        # row 32g is the softmax-denominator row (killed by zeros); V values
        # sit at rows 32g+1 .. 32g+16 (ones-column-first V-hat layout).
        wo_g = np.zeros((128, UNIF), np.float32)
        for g in range(HG):
            wo_g[32 * g + 1:32 * g + 1 + D, :] = Wo[:, 4 * grp + g, :]
        grp_consts.append((amat, wv_g, wo_g))

    in_maps = []
    for c in range(8):
        b, grp = c // 2, c % 2
        amat, wv_g, wo_g = grp_consts[grp]
        in_maps.append({
            "vecb": np.ascontiguousarray(vec[b]),
            "amat": amat,
            "wv": wv_g,
            "wo": wo_g,
            "sel": sel,
            "vinit": vinit,
            "ident": ident,
        })
    return in_maps


def _get_program():
    if "nc" not in _CACHE:
        _CACHE["nc"] = _build_program()
    return _CACHE["nc"]


def _run(inputs, trace=False, trace_kwargs=None):
    from concourse.bass_utils import run_bass_kernel_spmd

    nc = _get_program()
    in_maps = _prep_in_maps(**inputs)
    res = run_bass_kernel_spmd(
        nc, in_maps, core_ids=list(range(8)), trace=trace,
        **({"trace_kwargs": trace_kwargs} if trace_kwargs else {}),
    )
    _CACHE["last_results"] = res
    outs = [r["out"] for r in res.results]
    full = np.stack([outs[2 * b] + outs[2 * b + 1] for b in range(B)])
    return np.ascontiguousarray(full, np.float32)


def kernel(**inputs) -> np.ndarray:
    return _run(inputs, trace=False)


# revision 31
# speedup vs baseline: 1.6532x; 1.0778x over previous
# Multi-head attention kernel for Trainium2 (Bass/Tile), SPMD over 8 cores.
#
# Problem (hardcoded shapes):
#   Wq [128, 8, 16], Wk [128, 8, 16], Wv [128, 16, 8], Wo [16, 8, 128],
#   vec [4, 2048, 128]  ->  out [4, 2048, 128]   (all float32)
#
# Sharding: core c handles batch c//2 and head-group c%2 (4 heads each).
# Each core computes its 4 heads' contribution to the output projection;
# the host sums the two head-group partials per batch.
#
# Per-core layout choices:
#  - scores are computed transposed, St[j, i] (j on partitions), so that the
#    softmax denominator sum_j exp(s) falls out of the AV matmul via an extra
#    ones-column in V ("V-hat"), and no PSUM transposes are needed.
#  - head h of the group lives at partition offset 32*h (PE row tiling), so
#    2 score matmuls run concurrently in the 128x128 PE array despite
#    head_dim=16. AV matmuls accumulate into 4 per-head PSUM banks (f32r
#    requires dst partition 0 and even stationary width, hence VW=32).
#  - exp() on the scalar engine is the bottleneck (16.8M elems/core); it reads
#    score PSUM tiles [128, 1024] directly and writes SBUF, one pass.

import ml_dtypes
import numpy as np

B, N, UNIF, H, D = 4, 2048, 128, 8, 16
HG = 4         # heads per core
TI = 512       # i-tile width (query dim per inner tile)
TJ = 128       # j-tile width (key dim per matmul)
IT = N // TI   # 4 i-tiles
JT = N // TJ   # 16 j-tiles
VW = 32       # V-hat block width per head: col0=ones, 1..16=V, rest zero pad

_CACHE = {}


def _build_program():
    from contextlib import ExitStack

    import concourse.mybir as mybir
    import concourse.tile as tile
    from concourse import bacc

    f32 = mybir.dt.float32
    f32r = mybir.dt.float32r
    f16 = mybir.dt.float16
    AF = mybir.ActivationFunctionType

    nc = bacc.Bacc("TRN2", target_bir_lowering=False, debug=False)

    vecb = nc.dram_tensor("vecb", [N, UNIF], f32, kind="ExternalInput").ap()
    amat = nc.dram_tensor("amat", [128, HG * 128], f16, kind="ExternalInput").ap()
    wv = nc.dram_tensor("wv", [UNIF, HG * D], f16, kind="ExternalInput").ap()
    vinit = nc.dram_tensor("vinit", [128, JT * HG * VW], f16, kind="ExternalInput").ap()
    wo = nc.dram_tensor("wo", [128, UNIF], f32r, kind="ExternalInput").ap()
    sel = nc.dram_tensor("sel", [128, 128], f32r, kind="ExternalInput").ap()
    ident = nc.dram_tensor("ident", [128, 128], f32, kind="ExternalInput").ap()
    out = nc.dram_tensor("out", [N, UNIF], f32, kind="ExternalOutput").ap()

    with tile.TileContext(nc) as tc, ExitStack() as ctx:
        consts = ctx.enter_context(tc.tile_pool(name="consts", bufs=1))
        big = ctx.enter_context(tc.tile_pool(name="big", bufs=1))
        epool = ctx.enter_context(tc.tile_pool(name="epool", bufs=5))
        post = ctx.enter_context(tc.tile_pool(name="post", bufs=2))
        ps = ctx.enter_context(tc.tile_pool(name="ps", bufs=3, space="PSUM"))
        avp = ctx.enter_context(tc.tile_pool(name="avp", bufs=2, space="PSUM"))

        # ---- constants ----
        amat_s = consts.tile([128, HG * 128], f16)
        nc.sync.dma_start(out=amat_s, in_=amat)
        wv_s = consts.tile([128, HG * D], f16)
        nc.sync.dma_start(out=wv_s, in_=wv)
        wo_s = consts.tile([128, UNIF], f32r)
        nc.sync.dma_start(out=wo_s, in_=wo)
        sel_s = consts.tile([128, 128], f32r)
        nc.sync.dma_start(out=sel_s, in_=sel)
        id_s = consts.tile([128, 128], f32)
        nc.sync.dma_start(out=id_s, in_=ident)

        # ---- persistent SBUF tensors ----
        vec_in = big.tile([128, N], f32)            # vec rows tiled: [p][t*128+k]
        vecT = big.tile([128, N], f16)              # vec^T [k, n]
        ct = big.tile([128, HG * N], f16)           # Ct_g = (vec @ A_g)^T, [c, n]
        vhat = big.tile([128, JT * HG * VW], f16)   # [j%128][jt][g][32]; col 0 = ones
        vhat4 = vhat.rearrange("p (jt g e) -> p jt g e", jt=JT, g=HG)
        nc.sync.dma_start(out=vhat, in_=vinit)

        # ---- load vec (2 bulk DMAs) and transpose via PE (16x 128x128) ----
        vec3 = vec_in.rearrange("p (t k) -> p t k", k=TJ)
        vsrc = vecb.rearrange("(t p) k -> p t k", p=128)
        for half in range(2):
            nc.sync.dma_start(out=vec3[:, half * 8:(half + 1) * 8, :],
                              in_=vsrc[:, half * 8:(half + 1) * 8, :])
        for c4 in range(4):
            tp = ps.tile([128, 512], f32, tag="ps")
            for q in range(4):
                t = 4 * c4 + q
                nc.tensor.transpose(tp[:, q * 128:(q + 1) * 128], vec3[:, t, :], id_s)
            nc.vector.tensor_copy(out=vecT[:, c4 * 512:(c4 + 1) * 512], in_=tp)

        # ---- Ct_g = (vec @ A_g)^T and V, interleaved chunk-major so the
        #      first j-tiles' inputs are ready as early as possible
        for c4 in range(IT):
            for g in range(HG):
                cp = ps.tile([128, TI], f32, tag="ps", name="cp")
                nc.tensor.matmul(
                    cp,
                    lhsT=amat_s[:, g * 128:(g + 1) * 128],
                    rhs=vecT[:, c4 * TI:(c4 + 1) * TI],
                    start=True, stop=True,
                )
                nc.vector.tensor_copy(
                    out=ct[:, g * N + c4 * TI:g * N + (c4 + 1) * TI], in_=cp)
            for jt in range(4 * c4, 4 * c4 + 4):
                vp = ps.tile([128, HG * D], f32, tag="ps", name="vp")
                nc.tensor.matmul(
                    vp,
                    lhsT=vecT[:, jt * TJ:(jt + 1) * TJ],
                    rhs=wv_s,
                    start=True, stop=True,
                )
                nc.vector.tensor_copy(
                    out=vhat4[:, jt, :, 1:D + 1],
                    in_=vp.rearrange("p (g d) -> p g d", g=HG),
                )

        # ---- main attention loop (postlude deferred into the next i-tile
        #      so its PE ops never starve the scalar engine) ----
        post_a = [None]
        post_b = [None]

        def postlude_a(avt, it4):
            # drain the AV accumulator and broadcast the denominators;
            # returns state for postlude_b
            ot = post.tile([128, TI], f32r, tag="ot", name="ot")
            nc.vector.tensor_copy(out=ot, in_=avt)
            bb = ps.tile([128, TI], f32, tag="ps", name="bb")
            nc.tensor.matmul(bb, lhsT=sel_s, rhs=ot, start=True, stop=True)
            rec = post.tile([128, TI], f32, tag="rec", name="rec")
            nc.vector.reciprocal(out=rec, in_=bb)
            return (ot, rec)

        def postlude_b(state, it4):
            ot, rec = state
            otn = post.tile([128, TI], f32r, tag="otn", name="otn")
            nc.vector.tensor_mul(out=otn, in0=ot, in1=rec)
            for ic in range(4):
                fo = ps.tile([128, 128], f32, tag="ps", name="fo")
                nc.tensor.matmul(
                    fo,
                    lhsT=otn[:, ic * 128:(ic + 1) * 128],
                    rhs=wo_s,
                    start=True, stop=True,
                )
                ob = post.tile([128, 128], f32, tag="ob", name="ob")
                nc.vector.tensor_copy(out=ob, in_=fo)
                nc.sync.dma_start(
                    out=out[it4 * TI + ic * 128:it4 * TI + (ic + 1) * 128, :],
                    in_=ob,
                )

        for it4 in range(IT):
            avt = avp.tile([128, TI], f32, tag="av")
            nc.vector.memset(avt, 0.0)
            # software pipeline across j-tiles: emit scores(jt) then AV(jt-1)
            pend = None
            for jt in range(JT + 1):
                if jt < JT:
                    exs = []
                    for w in range(2):
                        sc = ps.tile([128, 2 * TI], f32, tag="ps", name=f"sc{w}")
                        for hh in range(2):
                            g = 2 * w + hh
                            nc.tensor.matmul(
                                sc[:, hh * TI:(hh + 1) * TI],
                                lhsT=ct[:, g * N + jt * TJ:g * N + (jt + 1) * TJ],
                                rhs=vecT[:, it4 * TI:(it4 + 1) * TI],
                                start=True, stop=True,
                            )
                        ex = epool.tile([128, 2 * TI], f16, tag="e", name=f"ex{w}")
                        nc.scalar.activation(out=ex, in_=sc, func=AF.Exp, scale=0.25)
                        exs.append(ex)
                if pend is not None:
                    pjt = jt - 1
                    for w in range(2):
                        for hh in range(2):
                            g = 2 * w + hh
                            nc.tensor.matmul(
                                avt[32 * g:32 * g + VW, :],
                                lhsT=vhat4[:, pjt, g, :],
                                rhs=pend[w][:, hh * TI:(hh + 1) * TI],
                                start=(pjt == 0 and g == 0), stop=(pjt == JT - 1),
                                tile_position=(0, 32 * g),
                                skip_group_check=(g > 0),
                            )
                pend = exs if jt < JT else None
                # flush the previous i-tile's postlude in two phases so the
                # reciprocal latency hides behind this i-tile's j-loop
                if jt == 0 and post_a[0] is not None:
                    post_b[0] = (post_a[0][0](*post_a[0][1]), post_a[0][2])
                    post_a[0] = None
                if jt == 5 and post_b[0] is not None:
                    postlude_b(*post_b[0])
                    post_b[0] = None
            post_a[0] = (postlude_a, (avt, it4), it4)
        st = post_a[0][0](*post_a[0][1])
        postlude_b(st, post_a[0][2])

    nc.compile()
    return nc


def _prep_in_maps(Wq, Wk, Wv, Wo, vec):
    Wq = np.ascontiguousarray(Wq, np.float32)
    Wk = np.ascontiguousarray(Wk, np.float32)
    Wv = np.ascontiguousarray(Wv, np.float32)
    Wo = np.ascontiguousarray(Wo, np.float32)
    vec = np.ascontiguousarray(vec, np.float32)

    # sel.T @ x broadcasts partition row 32*(m//32) of x to every row m of
    # that 32-row group (used to spread softmax denominators to their heads).
    sel = np.zeros((128, 128), np.float32)
    for m in range(128):
        sel[32 * (m // 32), m] = 1.0
    # V-hat static pattern: ones column at offset 0 of each 32-wide block
    vinit = np.zeros((128, JT * HG * VW), np.float32)
    vinit[:, ::VW] = 1.0
    vinit = vinit.astype(np.float16)
    ident = np.eye(128, dtype=np.float32)

    grp_consts = []
    for grp in range(2):
        hs = slice(4 * grp, 4 * grp + 4)
        # scores are computed as vec @ A_h @ vec^T with A_h = Wk_h Wq_h^T,
        # so S^T[j,i] = k_j . q_i  (precomputed on host in float64)
        amat = np.zeros((128, HG * 128), np.float32)
        for g in range(HG):
            h = 4 * grp + g
            A = Wk[:, h, :].astype(np.float64) @ Wq[:, h, :].astype(np.float64).T
            amat[:, g * 128:(g + 1) * 128] = A.astype(np.float32)
        # wv free order (g, d):  wv_g[k, 16g+d] = Wv[k, d, 4*grp+g]
        wv_g = np.ascontiguousarray(
            Wv[:, :, hs].transpose(0, 2, 1)).reshape(UNIF, HG * D)
        # row 32g is the softmax-denominator row (killed by zeros); V values
        # sit at rows 32g+1 .. 32g+16 (ones-column-first V-hat layout).
        wo_g = np.zeros((128, UNIF), np.float32)
        for g in range(HG):
            wo_g[32 * g + 1:32 * g + 1 + D, :] = Wo[:, 4 * grp + g, :]
        grp_consts.append((amat.astype(np.float16), wv_g.astype(np.float16), wo_g))

    in_maps = []
    for c in range(8):
        b, grp = c // 2, c % 2
        amat, wv_g, wo_g = grp_consts[grp]
        in_maps.append({
            "vecb": np.ascontiguousarray(vec[b]),
            "amat": amat,
            "wv": wv_g,
            "wo": wo_g,
            "sel": sel,
            "vinit": vinit,
            "ident": ident,
        })
    return in_maps


def _get_program():
    if "nc" not in _CACHE:
        _CACHE["nc"] = _build_program()
    return _CACHE["nc"]


def _run(inputs, trace=False, trace_kwargs=None):
    from concourse.bass_utils import run_bass_kernel_spmd

    nc = _get_program()
    in_maps = _prep_in_maps(**inputs)
    res = run_bass_kernel_spmd(
        nc, in_maps, core_ids=list(range(8)), trace=trace,
        **({"trace_kwargs": trace_kwargs} if trace_kwargs else {}),
    )
    _CACHE["last_results"] = res
    outs = [r["out"] for r in res.results]
    full = np.stack([outs[2 * b] + outs[2 * b + 1] for b in range(B)])
    return np.ascontiguousarray(full, np.float32)


def kernel(**inputs) -> np.ndarray:
    return _run(inputs, trace=False)


# revision 32
# speedup vs baseline: 1.6974x; 1.0268x over previous
# Multi-head attention kernel for Trainium2 (Bass/Tile), SPMD over 8 cores.
#
# Problem (hardcoded shapes):
#   Wq [128, 8, 16], Wk [128, 8, 16], Wv [128, 16, 8], Wo [16, 8, 128],
#   vec [4, 2048, 128]  ->  out [4, 2048, 128]   (all float32)
#
# Sharding: core c handles batch c//2 and head-group c%2 (4 heads each).
# Each core computes its 4 heads' contribution to the output projection;
# the host sums the two head-group partials per batch.
#
# Per-core layout choices:
#  - scores are computed transposed, St[j, i] (j on partitions), so that the
#    softmax denominator sum_j exp(s) falls out of the AV matmul via an extra
#    ones-column in V ("V-hat"), and no PSUM transposes are needed.
#  - head h of the group lives at partition offset 32*h (PE row tiling), so
#    2 score matmuls run concurrently in the 128x128 PE array despite
#    head_dim=16. AV matmuls accumulate into 4 per-head PSUM banks (f32r
#    requires dst partition 0 and even stationary width, hence VW=32).
#  - exp() on the scalar engine is the bottleneck (16.8M elems/core); it reads
#    score PSUM tiles [128, 1024] directly and writes SBUF, one pass.

import ml_dtypes
import numpy as np

B, N, UNIF, H, D = 4, 2048, 128, 8, 16
HG = 4         # heads per core
TI = 512       # i-tile width (query dim per inner tile)
TJ = 128       # j-tile width (key dim per matmul)
IT = N // TI   # 4 i-tiles
JT = N // TJ   # 16 j-tiles
VW = 32       # V-hat block width per head: col0=ones, 1..16=V, rest zero pad

_CACHE = {}


def _build_program():
    from contextlib import ExitStack

    import concourse.mybir as mybir
    import concourse.tile as tile
    from concourse import bacc

    f32 = mybir.dt.float32
    f32r = mybir.dt.float32r
    f16 = mybir.dt.float16
    AF = mybir.ActivationFunctionType

    nc = bacc.Bacc("TRN2", target_bir_lowering=False, debug=False)

    vecb = nc.dram_tensor("vecb", [N, UNIF], f32, kind="ExternalInput").ap()
    amat = nc.dram_tensor("amat", [128, HG * 128], f16, kind="ExternalInput").ap()
    wv = nc.dram_tensor("wv", [UNIF, HG * D], f16, kind="ExternalInput").ap()
    vinit = nc.dram_tensor("vinit", [128, JT * HG * VW], f16, kind="ExternalInput").ap()
    wo = nc.dram_tensor("wo", [128, UNIF], f32r, kind="ExternalInput").ap()
    sel = nc.dram_tensor("sel", [128, 128], f32r, kind="ExternalInput").ap()
    ident = nc.dram_tensor("ident", [128, 128], f32, kind="ExternalInput").ap()
    out = nc.dram_tensor("out", [N, UNIF], f32, kind="ExternalOutput").ap()

    with tile.TileContext(nc) as tc, ExitStack() as ctx:
        consts = ctx.enter_context(tc.tile_pool(name="consts", bufs=1))
        big = ctx.enter_context(tc.tile_pool(name="big", bufs=1))
        epool = ctx.enter_context(tc.tile_pool(name="epool", bufs=5))
        post = ctx.enter_context(tc.tile_pool(name="post", bufs=2))
        ps = ctx.enter_context(tc.tile_pool(name="ps", bufs=3, space="PSUM"))
        avp = ctx.enter_context(tc.tile_pool(name="avp", bufs=2, space="PSUM"))

        # ---- persistent SBUF tensors ----
        vec_in = big.tile([128, N], f32)            # vec rows tiled: [p][t*128+k]
        vecT = big.tile([128, N], f16)              # vec^T [k, n]
        ct = big.tile([128, HG * N], f16)           # Ct_g = (vec @ A_g)^T, [c, n]
        vhat = big.tile([128, JT * HG * VW], f16)   # [j%128][jt][g][32]; col 0 = ones
        vhat4 = vhat.rearrange("p (jt g e) -> p jt g e", jt=JT, g=HG)

        # ---- vec + identity first: they gate the transpose critical path ----
        vec3 = vec_in.rearrange("p (t k) -> p t k", k=TJ)
        vsrc = vecb.rearrange("(t p) k -> p t k", p=128)
        for quarter in range(4):
            nc.sync.dma_start(out=vec3[:, quarter * 4:(quarter + 1) * 4, :],
                              in_=vsrc[:, quarter * 4:(quarter + 1) * 4, :])
        id_s = consts.tile([128, 128], f32)
        nc.sync.dma_start(out=id_s, in_=ident)
        amat_s = consts.tile([128, HG * 128], f16)
        nc.sync.dma_start(out=amat_s, in_=amat)
        wv_s = consts.tile([128, HG * D], f16)
        nc.sync.dma_start(out=wv_s, in_=wv)
        nc.sync.dma_start(out=vhat, in_=vinit)
        wo_s = consts.tile([128, UNIF], f32r)
        nc.sync.dma_start(out=wo_s, in_=wo)
        sel_s = consts.tile([128, 128], f32r)
        nc.sync.dma_start(out=sel_s, in_=sel)

        # ---- transpose vec via PE (16x 128x128) ----
        for c4 in range(4):
            tp = ps.tile([128, 512], f32, tag="ps")
            for q in range(4):
                t = 4 * c4 + q
                nc.tensor.transpose(tp[:, q * 128:(q + 1) * 128], vec3[:, t, :], id_s)
            nc.vector.tensor_copy(out=vecT[:, c4 * 512:(c4 + 1) * 512], in_=tp)

        # ---- Ct_g = (vec @ A_g)^T and V, interleaved chunk-major so the
        #      first j-tiles' inputs are ready as early as possible
        for c4 in range(IT):
            for g in range(HG):
                cp = ps.tile([128, TI], f32, tag="ps", name="cp")
                nc.tensor.matmul(
                    cp,
                    lhsT=amat_s[:, g * 128:(g + 1) * 128],
                    rhs=vecT[:, c4 * TI:(c4 + 1) * TI],
                    start=True, stop=True,
                )
                if g % 2 == 0:
                    nc.vector.tensor_copy(
                        out=ct[:, g * N + c4 * TI:g * N + (c4 + 1) * TI], in_=cp)
                else:
                    nc.scalar.copy(
                        out=ct[:, g * N + c4 * TI:g * N + (c4 + 1) * TI], in_=cp)
            for jt in range(4 * c4, 4 * c4 + 4):
                vp = ps.tile([128, HG * D], f32, tag="ps", name="vp")
                nc.tensor.matmul(
                    vp,
                    lhsT=vecT[:, jt * TJ:(jt + 1) * TJ],
                    rhs=wv_s,
                    start=True, stop=True,
                )
                if jt % 2 == 0:
                    nc.vector.tensor_copy(
                        out=vhat4[:, jt, :, 1:D + 1],
                        in_=vp.rearrange("p (g d) -> p g d", g=HG),
                    )
                else:
                    nc.scalar.copy(
                        out=vhat4[:, jt, :, 1:D + 1],
                        in_=vp.rearrange("p (g d) -> p g d", g=HG),
                    )

        # ---- main attention loop (postlude deferred into the next i-tile
        #      so its PE ops never starve the scalar engine) ----
        post_a = [None]
        post_b = [None]

        def postlude_a(avt, it4):
            # drain the AV accumulator, broadcast denominators, and start the
            # reciprocal of the first column chunk
            ot = post.tile([128, TI], f32r, tag="ot", name="ot")
            nc.vector.tensor_copy(out=ot, in_=avt)
            bb = ps.tile([128, TI], f32, tag="ps", name="bb")
            nc.tensor.matmul(bb, lhsT=sel_s, rhs=ot, start=True, stop=True)
            rec = post.tile([128, TI], f32, tag="rec", name="rec")
            otn = post.tile([128, TI], f32r, tag="otn", name="otn")
            for ic in range(4):
                cs = slice(ic * 128, (ic + 1) * 128)
                nc.vector.reciprocal(out=rec[:, cs], in_=bb[:, cs])
                nc.vector.tensor_mul(out=otn[:, cs], in0=ot[:, cs], in1=rec[:, cs])
            return (otn,)

        def postlude_b(state, it4):
            (otn,) = state
            for ic in range(4):
                fo = ps.tile([128, 128], f32, tag="ps", name="fo")
                nc.tensor.matmul(
                    fo,
                    lhsT=otn[:, ic * 128:(ic + 1) * 128],
                    rhs=wo_s,
                    start=True, stop=True,
                )
                ob = post.tile([128, 128], f32, tag="ob", name="ob")
                nc.vector.tensor_copy(out=ob, in_=fo)
                nc.sync.dma_start(
                    out=out[it4 * TI + ic * 128:it4 * TI + (ic + 1) * 128, :],
                    in_=ob,
                )

        for it4 in range(IT):
            avt = avp.tile([128, TI], f32, tag="av")
            nc.vector.memset(avt, 0.0)
            # software pipeline across j-tiles: emit scores(jt) then AV(jt-1)
            pend = None
            for jt in range(JT + 1):
                if jt < JT:
                    exs = []
                    for w in range(2):
                        sc = ps.tile([128, 2 * TI], f32, tag="ps", name=f"sc{w}")
                        for hh in range(2):
                            g = 2 * w + hh
                            nc.tensor.matmul(
                                sc[:, hh * TI:(hh + 1) * TI],
                                lhsT=ct[:, g * N + jt * TJ:g * N + (jt + 1) * TJ],
                                rhs=vecT[:, it4 * TI:(it4 + 1) * TI],
                                start=True, stop=True,
                            )
                        ex = epool.tile([128, 2 * TI], f16, tag="e", name=f"ex{w}")
                        nc.scalar.activation(out=ex, in_=sc, func=AF.Exp, scale=0.25)
                        exs.append(ex)
                if pend is not None:
                    pjt = jt - 1
                    for w in range(2):
                        for hh in range(2):
                            g = 2 * w + hh
                            nc.tensor.matmul(
                                avt[32 * g:32 * g + VW, :],
                                lhsT=vhat4[:, pjt, g, :],
                                rhs=pend[w][:, hh * TI:(hh + 1) * TI],
                                start=(pjt == 0 and g == 0), stop=(pjt == JT - 1),
                                tile_position=(0, 32 * g),
                                skip_group_check=(g > 0),
                            )
                pend = exs if jt < JT else None
                # flush the previous i-tile's postlude in two phases so the
                # reciprocal latency hides behind this i-tile's j-loop
                if jt == 0 and post_a[0] is not None:
                    post_b[0] = (post_a[0][0](*post_a[0][1]), post_a[0][2])
                    post_a[0] = None
                if jt == 5 and post_b[0] is not None:
                    postlude_b(*post_b[0])
                    post_b[0] = None
            post_a[0] = (postlude_a, (avt, it4), it4)
        st = post_a[0][0](*post_a[0][1])
        postlude_b(st, post_a[0][2])

    nc.compile()
    return nc


def _prep_in_maps(Wq, Wk, Wv, Wo, vec):
    Wq = np.ascontiguousarray(Wq, np.float32)
    Wk = np.ascontiguousarray(Wk, np.float32)
    Wv = np.ascontiguousarray(Wv, np.float32)
    Wo = np.ascontiguousarray(Wo, np.float32)
    vec = np.ascontiguousarray(vec, np.float32)

    # sel.T @ x broadcasts partition row 32*(m//32) of x to every row m of
    # that 32-row group (used to spread softmax denominators to their heads).
    sel = np.zeros((128, 128), np.float32)
    for m in range(128):
        sel[32 * (m // 32), m] = 1.0
    # V-hat static pattern: ones column at offset 0 of each 32-wide block
    vinit = np.zeros((128, JT * HG * VW), np.float32)
    vinit[:, ::VW] = 1.0
    vinit = vinit.astype(np.float16)
    ident = np.eye(128, dtype=np.float32)

    grp_consts = []
    for grp in range(2):
        hs = slice(4 * grp, 4 * grp + 4)
        # scores are computed as vec @ A_h @ vec^T with A_h = Wk_h Wq_h^T,
        # so S^T[j,i] = k_j . q_i  (precomputed on host in float64)
        amat = np.zeros((128, HG * 128), np.float32)
        for g in range(HG):
            h = 4 * grp + g
            A = Wk[:, h, :].astype(np.float64) @ Wq[:, h, :].astype(np.float64).T
            amat[:, g * 128:(g + 1) * 128] = A.astype(np.float32)
        # wv free order (g, d):  wv_g[k, 16g+d] = Wv[k, d, 4*grp+g]
        wv_g = np.ascontiguousarray(
            Wv[:, :, hs].transpose(0, 2, 1)).reshape(UNIF, HG * D)
        # row 32g is the softmax-denominator row (killed by zeros); V values
        # sit at rows 32g+1 .. 32g+16 (ones-column-first V-hat layout).
        wo_g = np.zeros((128, UNIF), np.float32)
        for g in range(HG):
            wo_g[32 * g + 1:32 * g + 1 + D, :] = Wo[:, 4 * grp + g, :]
        grp_consts.append((amat.astype(np.float16), wv_g.astype(np.float16), wo_g))

    in_maps = []
    for c in range(8):
        b, grp = c // 2, c % 2
        amat, wv_g, wo_g = grp_consts[grp]
        in_maps.append({
            "vecb": np.ascontiguousarray(vec[b]),
            "amat": amat,
            "wv": wv_g,
            "wo": wo_g,
            "sel": sel,
            "vinit": vinit,
            "ident": ident,
        })
    return in_maps


def _get_program():
    if "nc" not in _CACHE:
        _CACHE["nc"] = _build_program()
    return _CACHE["nc"]


def _run(inputs, trace=False, trace_kwargs=None):
    from concourse.bass_utils import run_bass_kernel_spmd

    nc = _get_program()
    in_maps = _prep_in_maps(**inputs)
    res = run_bass_kernel_spmd(
        nc, in_maps, core_ids=list(range(8)), trace=trace,
        **({"trace_kwargs": trace_kwargs} if trace_kwargs else {}),
    )
    _CACHE["last_results"] = res
    outs = [r["out"] for r in res.results]
    full = np.stack([outs[2 * b] + outs[2 * b + 1] for b in range(B)])
    return np.ascontiguousarray(full, np.float32)


def kernel(**inputs) -> np.ndarray:
    return _run(inputs, trace=False)
